# revision 13
# baseline (speedup 1.0000x reference)
"""Trainium2 raw-Bass kernel: 3x3 same-padding conv2d, 64->64 channels, on
x(16,64,112,112) f32, data-parallel over batch across 8 NeuronCores.

Per core (2 images), hand-scheduled raw Bass (no TileContext):
  - Host pre-pads each image to 114x114 zeros; input is one contiguous
    [128, 13000] bf16 region (partitions 0-63 = image0 cin, 64-127 =
    image1 cin); every conv tap is a flat offset slice of the SBUF tile.
  - Conv = 9 accumulated matmuls per 4-row block (K=cin=64, M=cout=64,
    N=456) with PE-array quadrant packing: 4 independent 64x64 matmuls
    stream concurrently (2 images x 2 adjacent row-blocks).
  - PSUM -> SBUF drain fused with bias add (scalar engine for the A
    half, vector engine for the B half), output staged in SBUF as bf16
    and DMA'd out in chunks on both HWDGE rings; host upcasts to f32.
  - Manual semaphore protocol (7 sems). The Bass preamble zeroes the
    kernel sem range before engine programs start, and the kernel runs
    once per NEFF load, so no teardown clears are emitted (this removes
    the ~7us per-semaphore clear tail the Tile framework generates).
"""

import numpy as np
import ml_dtypes

import concourse.bacc as bacc
import concourse.mybir as mybir
from concourse import bass_utils

FP32 = mybir.dt.float32
BF16 = mybir.dt.bfloat16

P = 128          # SBUF partitions
CIN = 64
COUT = 64
H = W = 112
Wp = W + 2       # padded width
Hp = H + 2
NROW = 4         # output rows per matmul block
NBLK = NROW * Wp  # matmul free size = 456
G = 14           # row-block pairs (8 rows per group)
XS_LEN = Hp * Wp + 4   # 12996 + slack for tap-offset overrun
OUT_LEN = G * NBLK     # 6384 per half

TAPS = [(kh, kw) for kh in range(3) for kw in range(3)]
XHDR = 9 * COUT + 2   # header cols in xin: 576 weights + 2 (f32 bias bits)
# input chunk boundaries (flat cols); chunk0 small so group 0 un-gates fast
IN_BOUNDS = [0, 1152, 3306, 8151, XS_LEN]
# output chunks (group ranges), triggered after the drain of their last group
OUT_CHUNKS = [(0, 2), (2, 4), (4, 6), (6, 8), (8, 10), (10, 12), (12, 13),
              (13, 14)]


def _chunks_needed(g):
    """How many input chunks must have landed before group g's matmuls."""
    need = (8 * g + 6) * Wp + 2 + NBLK  # max flat index read + 1
    for c in range(1, len(IN_BOUNDS)):
        if IN_BOUNDS[c] >= need:
            return c
    raise AssertionError(need)


def _build_nc(n_cores: int = 8):
    nc = bacc.Bacc("TRN2", target_bir_lowering=False, debug=False,
                   num_devices=n_cores)
    x_d = nc.dram_tensor("xin", (P, XHDR + XS_LEN), BF16,
                         kind="ExternalInput").ap()
    y_d = nc.dram_tensor("yout", (2, P, OUT_LEN), BF16,
                         kind="ExternalOutput").ap()

    import contextlib
    ctx = contextlib.ExitStack()
    xst = ctx.enter_context(nc.sbuf_tensor("xs", [P, XHDR + XS_LEN], BF16))
    xs = xst[:, XHDR:]             # padded image region
    wsb = xst[:, 0:9 * COUT]       # weights header
    bsb = xst[:, 9 * COUT:XHDR].bitcast(FP32)   # f32 bias bits
    osbA = ctx.enter_context(nc.sbuf_tensor("osbA", [P, OUT_LEN], BF16))
    osbB = ctx.enter_context(nc.sbuf_tensor("osbB", [P, OUT_LEN], BF16))
    # 8 PSUM banks: psA rotates banks 0-3, psB banks 4-7 ([128,512] = 1 bank)
    pa = [ctx.enter_context(nc.psum_tensor(f"pa{i}", [P, 512], FP32))
          for i in range(4)]
    pb = [ctx.enter_context(nc.psum_tensor(f"pb{i}", [P, 512], FP32))
          for i in range(4)]

    # semaphores (never released: keeps indexes stable; preamble zeroes them)
    sIn = nc.alloc_semaphore("sIn")     # q1 input chunk completions (+16)
    sW = nc.alloc_semaphore("sW")       # q10 weights+bias completions (+16)
    sMM = nc.alloc_semaphore("sMM")     # tensor: +1 per tap8 matmul (4/group)
    sDrA = nc.alloc_semaphore("sDrA")   # scalar: +1 per A drain
    sDrB = nc.alloc_semaphore("sDrB")   # vector: +1 per B drain
    sOutA = nc.alloc_semaphore("sOutA")  # q10 output chunk completions (+16)
    sOutB = nc.alloc_semaphore("sOutB")  # q1 output chunk completions (+16)

    with nc.Block("conv") as block:

        @block.sync
        def _(sync):
            # One header DMA (weights + bias bits) leads the SP HWDGE ring
            # (q1 starts pumping ~1.5us before q10, whose early descriptors
            # sit behind the act/dve table loads), then the input chunks.
            # q1 FIFO completion order makes one semaphore sufficient, and
            # one trigger instruction (~0.65us each) covers wt+bias.
            sync.dma_start(xst[:, 0:XHDR], x_d[:, 0:XHDR]).then_inc(sIn, 16)
            for c0, c1 in zip(IN_BOUNDS, IN_BOUNDS[1:]):
                sync.dma_start(xst[:, XHDR + c0:XHDR + c1],
                               x_d[:, XHDR + c0:XHDR + c1]).then_inc(sIn, 16)
            # output B chunks, gated on vector drains
            for i, (g0, g1) in enumerate(OUT_CHUNKS):
                s0, s1 = g0 * NBLK, g1 * NBLK
                sync.wait_ge(sDrB, g1)
                sync.dma_start(y_d[1, :, s0:s1],
                               osbB[:, s0:s1]).then_inc(sOutB, 16)
            sync.wait_ge(sOutB, 16 * len(OUT_CHUNKS))

        @block.scalar
        def _(scalar):
            # PSUM->SBUF drain of the A half with fused bias add (bf16 out),
            # plus the A output chunks on the ACT HWDGE ring (q10).
            scalar.wait_ge(sIn, 16)  # header (bias) resident
            for g in range(G):
                scalar.wait_ge(sMM, 4 * g + 4)
                scalar.add(osbA[:, g * NBLK:(g + 1) * NBLK],
                           pa[g % 4][:, :NBLK], bsb[:, 0:1]).then_inc(sDrA, 1)
                for g0, g1 in OUT_CHUNKS:
                    if g1 == g + 1:
                        s0, s1 = g0 * NBLK, g1 * NBLK
                        scalar.dma_start(y_d[0, :, s0:s1],
                                         osbA[:, s0:s1]).then_inc(sOutA, 16)
            scalar.wait_ge(sOutA, 16 * len(OUT_CHUNKS))

        @block.vector
        def _(vector):
            vector.wait_ge(sIn, 16)  # header (bias) resident
            for g in range(G):
                vector.wait_ge(sMM, 4 * g + 4)
                vector.tensor_scalar_add(osbB[:, g * NBLK:(g + 1) * NBLK],
                                         pb[g % 4][:, :NBLK],
                                         bsb[:, 0:1]).then_inc(sDrB, 1)

        @block.tensor
        def _(tensor):
            tensor.wait_ge(sIn, 16)  # header (weights) resident
            prev_chunks = 0
            for g in range(G):
                need = _chunks_needed(g)
                if need > prev_chunks:
                    tensor.wait_ge(sIn, 16 * (need + 1))  # +header
                    prev_chunks = need
                if g >= 4:
                    tensor.wait_ge(sDrA, g - 3)
                    tensor.wait_ge(sDrB, g - 3)
                psA = pa[g % 4]
                psB = pb[g % 4]
                rA = 8 * g
                rB = 8 * g + 4
                for t, (kh, kw) in enumerate(TAPS):
                    st = t == 0
                    sp = t == 8
                    w0 = wsb[0:64, t * 64:(t + 1) * 64]
                    w1 = wsb[64:128, t * 64:(t + 1) * 64]
                    oA = (rA + kh) * Wp + kw
                    oB = (rB + kh) * Wp + kw
                    # 4 concurrent PE-quadrant matmuls: (row_grp, col_grp)
                    m1 = tensor.matmul(psA[0:64, :NBLK], w0,
                                       xs[0:64, oA:oA + NBLK],
                                       start=st, stop=sp, tile_position=(0, 0))
                    m2 = tensor.matmul(psA[64:128, :NBLK], w1,
                                       xs[64:128, oA:oA + NBLK],
                                       start=st, stop=sp,
                                       tile_position=(64, 64))
                    m3 = tensor.matmul(psB[0:64, :NBLK], w1,
                                       xs[64:128, oB:oB + NBLK],
                                       start=st, stop=sp,
                                       tile_position=(64, 0))
                    m4 = tensor.matmul(psB[64:128, :NBLK], w0,
                                       xs[0:64, oB:oB + NBLK],
                                       start=st, stop=sp,
                                       tile_position=(0, 64))
                    if sp:
                        for m in (m1, m2, m3, m4):
                            m.then_inc(sMM, 1)

    nc.compile()
    ctx.close()
    return nc


_NC = None


def _get_nc():
    global _NC
    if _NC is None:
        _NC = _build_nc()
    return _NC


def _prep_in_maps(x, weights, bias, n_cores=8):
    # lhsT per tap: wt[cin, t*64+cout] = weights[cout, cin, kh, kw],
    # replicated into both partition halves.
    tmp = np.ascontiguousarray(
        weights.astype(np.float32).transpose(2, 3, 1, 0)).reshape(9, CIN, COUT)
    wt = np.empty((P, 9 * COUT), ml_dtypes.bfloat16)
    wt[0:64] = tmp.transpose(1, 0, 2).reshape(CIN, 9 * COUT)
    wt[64:128] = wt[0:64]
    bs = np.tile(np.asarray(bias, np.float32), 2).reshape(P, 1)

    xb = np.asarray(x, np.float32).astype(ml_dtypes.bfloat16)
    # [header: wt(576) | bias f32 bits(2)] then pre-padded image region
    xp = np.zeros((n_cores, P, XHDR + XS_LEN), ml_dtypes.bfloat16)
    xp[:, :, 0:9 * COUT] = wt
    xp[:, :, 9 * COUT:XHDR] = np.ascontiguousarray(bs).view(ml_dtypes.bfloat16)
    interior = xp[:, :, XHDR:XHDR + Hp * Wp].reshape(n_cores, P, Hp, Wp)
    interior[:, :, 1:1 + H, 1:1 + W] = xb.reshape(n_cores, P, H, W)
    in_maps = []
    for i in range(n_cores):
        in_maps.append({"xin": xp[i]})
    return in_maps


def _assemble(yout):
    # yout: [2, 128, 6384] bf16 -> (2, 64, 112, 112) f32 for this core.
    y = yout.astype(np.float32).reshape(2, 2, 64, G, NROW, Wp)[:, :, :, :, :, :W]
    out = np.empty((2, 64, G, 8, W), np.float32)
    out[0, :, :, 0:4] = y[0, 0]   # osbA[0:64]   = img0 rows 8g..8g+4
    out[1, :, :, 0:4] = y[0, 1]   # osbA[64:128] = img1 rows 8g..8g+4
    out[0, :, :, 4:8] = y[1, 1]   # osbB[64:128] = img0 rows 8g+4..8g+8
    out[1, :, :, 4:8] = y[1, 0]   # osbB[0:64]   = img1 rows 8g+4..8g+8
    return out.reshape(2, 64, H, W)


def kernel(x, weights, bias, _trace=False, _tmpdir=None):
    nc = _get_nc()
    in_maps = _prep_in_maps(x, weights, bias)
    res = bass_utils.run_bass_kernel_spmd(nc, in_maps,
                                          core_ids=list(range(8)),
                                          trace=_trace, tmpdir=_tmpdir)
    out = np.concatenate([_assemble(res.results[i]["yout"])
                          for i in range(8)], axis=0)
    if _trace:
        return out, res
    return out


# revision 15
# speedup vs baseline: 1.0097x; 1.0097x over previous
"""Trainium2 raw-Bass kernel: 3x3 same-padding conv2d, 64->64 channels, on
x(16,64,112,112) f32, data-parallel over batch across 8 NeuronCores.

Per core (2 images), hand-scheduled raw Bass (no TileContext):
  - Host pre-pads each image to 114x114 zeros; input is one contiguous
    [128, 13000] bf16 region (partitions 0-63 = image0 cin, 64-127 =
    image1 cin); every conv tap is a flat offset slice of the SBUF tile.
  - Conv = 9 accumulated matmuls per 4-row block (K=cin=64, M=cout=64,
    N=456) with PE-array quadrant packing: 4 independent 64x64 matmuls
    stream concurrently (2 images x 2 adjacent row-blocks).
  - PSUM -> SBUF drain fused with bias add (scalar engine for the A
    half, vector engine for the B half), output staged in SBUF as bf16
    and DMA'd out in chunks on both HWDGE rings; host upcasts to f32.
  - Manual semaphore protocol (7 sems). The Bass preamble zeroes the
    kernel sem range before engine programs start, and the kernel runs
    once per NEFF load, so no teardown clears are emitted (this removes
    the ~7us per-semaphore clear tail the Tile framework generates).
"""

import numpy as np
import ml_dtypes

import concourse.bacc as bacc
import concourse.mybir as mybir
from concourse import bass_utils

FP32 = mybir.dt.float32
BF16 = mybir.dt.bfloat16

P = 128          # SBUF partitions
CIN = 64
COUT = 64
H = W = 112
Wp = W + 2       # padded width
Hp = H + 2
NROW = 4         # output rows per matmul block
NBLK = NROW * Wp  # matmul free size = 456
G = 14           # row-block pairs (8 rows per group)
XS_LEN = Hp * Wp + 4   # 12996 + slack for tap-offset overrun
OUT_LEN = G * NBLK     # 6384 per half

TAPS = [(kh, kw) for kh in range(3) for kw in range(3)]
XHDR = 9 * COUT + 2   # header cols in xin: 576 weights + 2 (f32 bias bits)
# input chunk boundaries (flat cols); chunk0 small so group 0 un-gates fast
IN_BOUNDS = [0, 1152, 3306, 8151, XS_LEN]
# output chunks (group ranges), triggered after the drain of their last group
OUT_CHUNKS = [(0, 2), (2, 4), (4, 6), (6, 8), (8, 10), (10, 12), (12, 13),
              (13, 14)]


def _chunks_needed(g):
    """How many input chunks must have landed before group g's matmuls."""
    need = (8 * g + 6) * Wp + 2 + NBLK  # max flat index read + 1
    for c in range(1, len(IN_BOUNDS)):
        if IN_BOUNDS[c] >= need:
            return c
    raise AssertionError(need)


def _build_nc(n_cores: int = 8):
    nc = bacc.Bacc("TRN2", target_bir_lowering=False, debug=False,
                   num_devices=n_cores)
    x_d = nc.dram_tensor("xin", (P, XHDR + XS_LEN), BF16,
                         kind="ExternalInput").ap()
    y_d = nc.dram_tensor("yout", (2, P, OUT_LEN), BF16,
                         kind="ExternalOutput").ap()

    import contextlib
    ctx = contextlib.ExitStack()
    xst = ctx.enter_context(nc.sbuf_tensor("xs", [P, XHDR + XS_LEN], BF16))
    xs = xst[:, XHDR:]             # padded image region
    wsb = xst[:, 0:9 * COUT]       # weights header
    bsb = xst[:, 9 * COUT:XHDR].bitcast(FP32)   # f32 bias bits
    osbA = ctx.enter_context(nc.sbuf_tensor("osbA", [P, OUT_LEN], BF16))
    osbB = ctx.enter_context(nc.sbuf_tensor("osbB", [P, OUT_LEN], BF16))
    # 8 PSUM banks: psA rotates banks 0-3, psB banks 4-7 ([128,512] = 1 bank)
    pa = [ctx.enter_context(nc.psum_tensor(f"pa{i}", [P, 512], FP32))
          for i in range(4)]
    pb = [ctx.enter_context(nc.psum_tensor(f"pb{i}", [P, 512], FP32))
          for i in range(4)]

    # semaphores (never released: keeps indexes stable; preamble zeroes them)
    sIn = nc.alloc_semaphore("sIn")     # q1 input chunk completions (+16)
    sW = nc.alloc_semaphore("sW")       # q10 weights+bias completions (+16)
    sMM = nc.alloc_semaphore("sMM")     # tensor: +1 per tap8 matmul (4/group)
    sDrA = nc.alloc_semaphore("sDrA")   # scalar: +1 per A drain
    sDrB = nc.alloc_semaphore("sDrB")   # vector: +1 per B drain
    sOutA = nc.alloc_semaphore("sOutA")  # q10 output chunk completions (+16)
    sOutB = nc.alloc_semaphore("sOutB")  # q1 output chunk completions (+16)

    with nc.Block("conv") as block:

        @block.gpsimd
        def _(gpsimd):
            # weights+bias header via the SWDGE queue: gpsimd is otherwise
            # idle, starts right after the preamble, and q0 is not stuck
            # behind the act/dve table loads like q10's early descriptors.
            gpsimd.dma_start(xst[:, 0:XHDR], x_d[:, 0:XHDR]).then_inc(sW, 16)

        @block.sync
        def _(sync):
            # input chunks on the SP HWDGE ring; chunk0 leads so group 0
            # un-gates as early as possible.
            for c0, c1 in zip(IN_BOUNDS, IN_BOUNDS[1:]):
                sync.dma_start(xst[:, XHDR + c0:XHDR + c1],
                               x_d[:, XHDR + c0:XHDR + c1]).then_inc(sIn, 16)
            # output B chunks, gated on vector drains
            for i, (g0, g1) in enumerate(OUT_CHUNKS):
                s0, s1 = g0 * NBLK, g1 * NBLK
                sync.wait_ge(sDrB, g1)
                sync.dma_start(y_d[1, :, s0:s1],
                               osbB[:, s0:s1]).then_inc(sOutB, 16)
            sync.wait_ge(sOutB, 16 * len(OUT_CHUNKS))

        @block.scalar
        def _(scalar):
            # PSUM->SBUF drain of the A half with fused bias add (bf16 out),
            # plus the A output chunks on the ACT HWDGE ring (q10).
            scalar.wait_ge(sW, 16)  # header (bias) resident
            for g in range(G):
                scalar.wait_ge(sMM, 4 * g + 4)
                scalar.add(osbA[:, g * NBLK:(g + 1) * NBLK],
                           pa[g % 4][:, :NBLK], bsb[:, 0:1]).then_inc(sDrA, 1)
                for g0, g1 in OUT_CHUNKS:
                    if g1 == g + 1:
                        s0, s1 = g0 * NBLK, g1 * NBLK
                        scalar.dma_start(y_d[0, :, s0:s1],
                                         osbA[:, s0:s1]).then_inc(sOutA, 16)
            scalar.wait_ge(sOutA, 16 * len(OUT_CHUNKS))

        @block.vector
        def _(vector):
            vector.wait_ge(sW, 16)  # header (bias) resident
            for g in range(G):
                vector.wait_ge(sMM, 4 * g + 4)
                vector.tensor_scalar_add(osbB[:, g * NBLK:(g + 1) * NBLK],
                                         pb[g % 4][:, :NBLK],
                                         bsb[:, 0:1]).then_inc(sDrB, 1)

        @block.tensor
        def _(tensor):
            tensor.wait_ge(sW, 16)  # header (weights) resident
            prev_chunks = 0
            for g in range(G):
                need = _chunks_needed(g)
                if need > prev_chunks:
                    tensor.wait_ge(sIn, 16 * need)
                    prev_chunks = need
                if g >= 4:
                    tensor.wait_ge(sDrA, g - 3)
                    tensor.wait_ge(sDrB, g - 3)
                psA = pa[g % 4]
                psB = pb[g % 4]
                rA = 8 * g
                rB = 8 * g + 4
                for t, (kh, kw) in enumerate(TAPS):
                    st = t == 0
                    sp = t == 8
                    w0 = wsb[0:64, t * 64:(t + 1) * 64]
                    w1 = wsb[64:128, t * 64:(t + 1) * 64]
                    oA = (rA + kh) * Wp + kw
                    oB = (rB + kh) * Wp + kw
                    # 4 concurrent PE-quadrant matmuls: (row_grp, col_grp)
                    m1 = tensor.matmul(psA[0:64, :NBLK], w0,
                                       xs[0:64, oA:oA + NBLK],
                                       start=st, stop=sp, tile_position=(0, 0))
                    m2 = tensor.matmul(psA[64:128, :NBLK], w1,
                                       xs[64:128, oA:oA + NBLK],
                                       start=st, stop=sp,
                                       tile_position=(64, 64))
                    m3 = tensor.matmul(psB[0:64, :NBLK], w1,
                                       xs[64:128, oB:oB + NBLK],
                                       start=st, stop=sp,
                                       tile_position=(64, 0))
                    m4 = tensor.matmul(psB[64:128, :NBLK], w0,
                                       xs[0:64, oB:oB + NBLK],
                                       start=st, stop=sp,
                                       tile_position=(0, 64))
                    if sp:
                        for m in (m1, m2, m3, m4):
                            m.then_inc(sMM, 1)

    nc.compile()
    ctx.close()
    return nc


_NC = None


def _get_nc():
    global _NC
    if _NC is None:
        _NC = _build_nc()
    return _NC


def _prep_in_maps(x, weights, bias, n_cores=8):
    # lhsT per tap: wt[cin, t*64+cout] = weights[cout, cin, kh, kw],
    # replicated into both partition halves.
    tmp = np.ascontiguousarray(
        weights.astype(np.float32).transpose(2, 3, 1, 0)).reshape(9, CIN, COUT)
    wt = np.empty((P, 9 * COUT), ml_dtypes.bfloat16)
    wt[0:64] = tmp.transpose(1, 0, 2).reshape(CIN, 9 * COUT)
    wt[64:128] = wt[0:64]
    bs = np.tile(np.asarray(bias, np.float32), 2).reshape(P, 1)

    xb = np.asarray(x, np.float32).astype(ml_dtypes.bfloat16)
    # [header: wt(576) | bias f32 bits(2)] then pre-padded image region
    xp = np.zeros((n_cores, P, XHDR + XS_LEN), ml_dtypes.bfloat16)
    xp[:, :, 0:9 * COUT] = wt
    xp[:, :, 9 * COUT:XHDR] = np.ascontiguousarray(bs).view(ml_dtypes.bfloat16)
    interior = xp[:, :, XHDR:XHDR + Hp * Wp].reshape(n_cores, P, Hp, Wp)
    interior[:, :, 1:1 + H, 1:1 + W] = xb.reshape(n_cores, P, H, W)
    in_maps = []
    for i in range(n_cores):
        in_maps.append({"xin": xp[i]})
    return in_maps


def _assemble(yout):
    # yout: [2, 128, 6384] bf16 -> (2, 64, 112, 112) f32 for this core.
    y = yout.astype(np.float32).reshape(2, 2, 64, G, NROW, Wp)[:, :, :, :, :, :W]
    out = np.empty((2, 64, G, 8, W), np.float32)
    out[0, :, :, 0:4] = y[0, 0]   # osbA[0:64]   = img0 rows 8g..8g+4
    out[1, :, :, 0:4] = y[0, 1]   # osbA[64:128] = img1 rows 8g..8g+4
    out[0, :, :, 4:8] = y[1, 1]   # osbB[64:128] = img0 rows 8g+4..8g+8
    out[1, :, :, 4:8] = y[1, 0]   # osbB[0:64]   = img1 rows 8g+4..8g+8
    return out.reshape(2, 64, H, W)


def kernel(x, weights, bias, _trace=False, _tmpdir=None):
    nc = _get_nc()
    in_maps = _prep_in_maps(x, weights, bias)
    res = bass_utils.run_bass_kernel_spmd(nc, in_maps,
                                          core_ids=list(range(8)),
                                          trace=_trace, tmpdir=_tmpdir)
    out = np.concatenate([_assemble(res.results[i]["yout"])
                          for i in range(8)], axis=0)
    if _trace:
        return out, res
    return out
